# revision 10
# baseline (speedup 1.0000x reference)
"""Trainium2 Bass kernel for column self-attention (nn_ColumnSelfAttention).

Reference computation (per column c, columns are independent attention
problems):
    q = (x @ Wq + bq) * head_dim**-0.5 ; k = x @ Wk + bk ; v = x @ Wv + bv
    scores[h,c,i,j] = sum_d q[i,c,h,d] k[j,c,h,d]
    scores = where(mask[j,c], scores, -1e4); p = softmax_j(scores)
    ctx[i,c,:] = concat_h(p @ v) ; out = ctx @ Wo + bo

Sharding: the 256 columns are split across 8 NeuronCores (32 each).
Per core, tokens are ordered (column-major, row-inner) so one 128-token
tile == one column.  All matmul inputs are fp16 (fp32 PSUM accumulation);
softmax runs in fp32 on the scalar/vector engines.

Layout trick: scores are computed transposed (S_T[j,i]) so the key mask
becomes a per-partition bias fused into the Exp activation, and the
softmax denominator comes for free from an appended ones-column on V
(out[:, 64] of each head block = row sums).  The only transpose needed
is x (done by DMA-transpose on load) and the normalized context (PE
transpose via identity matmul) before the output projection.
"""

import os
import numpy as np

import concourse.bacc as bacc
import concourse.tile as tile
import concourse.mybir as mybir
from concourse import bass
from concourse.bass_utils import run_bass_kernel_spmd

R, C, E, H, D = 128, 256, 768, 12, 64
NCORES = 8
CLOC = C // NCORES            # 32 columns per core
BLK = 4                       # columns per block
NBLK = CLOC // BLK
T = BLK * R                   # 512 tokens per block
NTOK = CLOC * R               # 4096 tokens per core
NCH = E // 128                # 6 chunks of the embedding dim
F16 = mybir.dt.float16
F32 = mybir.dt.float32
Act = mybir.ActivationFunctionType

LAST_RESULTS = None           # for test.py introspection


def build_program(with_bias: bool, nblocks: int = NBLK, stage: int = 8):
    nc = bacc.Bacc("TRN2", target_bir_lowering=False, debug=False)

    # x^T per core, host-pretransposed: x_d[ec, p, t] = x[t, ec*128+p]
    x_d = nc.dram_tensor("x", [NCH, 128, NTOK], F16, kind="ExternalInput")
    madd_d = nc.dram_tensor("madd", [R, CLOC], F32, kind="ExternalInput")
    ident_d = nc.dram_tensor("ident", [128, 128], F16, kind="ExternalInput")
    w_d = {
        n: nc.dram_tensor(n, [E, E], F16, kind="ExternalInput")
        for n in ("wq", "wk", "wv", "wo")
    }
    if with_bias:
        bqk_d = {
            n: nc.dram_tensor(n, [128, NCH], F32, kind="ExternalInput")
            for n in ("bq", "bk")
        }
        bvo_d = {
            n: nc.dram_tensor(n, [1, E], F16, kind="ExternalInput")
            for n in ("bv", "bo")
        }
    o_d = nc.dram_tensor("o", [nblocks * BLK, R, E], F32, kind="ExternalOutput")

    with tile.TileContext(nc) as tc:
        with (
            tc.tile_pool(name="const", bufs=1) as const,
            tc.tile_pool(name="blk", bufs=2) as blkp,
            tc.tile_pool(name="col", bufs=3) as colp,
            tc.tile_pool(name="psmm", bufs=3, space="PSUM") as psmm,
            tc.tile_pool(name="pss", bufs=3, space="PSUM") as pssp,
            tc.tile_pool(name="pscx", bufs=2, space="PSUM") as pscx,
        ):
            w_sb = {}
            for n in ("wq", "wk", "wv", "wo"):
                w_sb[n] = const.tile([128, NCH, E], F16, tag=n, name=f"w_{n}")
                nc.gpsimd.dma_start(
                    w_sb[n][:], w_d[n].ap().rearrange("(c p) e -> p c e", p=128)
                )
            madd_sb = const.tile([R, CLOC], F32, tag="madd")
            nc.gpsimd.dma_start(madd_sb[:], madd_d.ap())
            ident_sb = const.tile([128, 128], F16, tag="ident")
            nc.gpsimd.dma_start(ident_sb[:], ident_d.ap())
            if with_bias:
                bqk_sb = {}
                for n in ("bq", "bk"):
                    bqk_sb[n] = const.tile([128, NCH], F32, tag=n, name=f"b_{n}")
                    nc.gpsimd.dma_start(bqk_sb[n][:], bqk_d[n].ap())
                bvo_sb = {}
                for n in ("bv", "bo"):
                    bvo_sb[n] = const.tile([1, E], F16, tag=n, name=f"b_{n}")
                    nc.gpsimd.dma_start(bvo_sb[n][:], bvo_d[n].ap())
                ones_sb = const.tile([1, 128], F16, tag="ones")
                nc.gpsimd.memset(ones_sb[:], 1.0)

            for b in range(nblocks):
                # ---- x^T for this block: (e, tok), plain contiguous DMA ----
                xt = blkp.tile([128, NCH, T], F16, tag="xt")
                for ec in range(NCH):
                    nc.sync.dma_start(
                        out=xt[:, ec, :],
                        in_=x_d.ap()[ec, :, b * T : (b + 1) * T],
                    )

                if stage < 2:
                    continue
                # ---- Q^T, K^T projections: (e_out, tok) ----
                qt = blkp.tile([128, NCH, T], F16, tag="qt")
                kt = blkp.tile([128, NCH, T], F16, tag="kt")
                for wname, bname, dst in (("wq", "bq", qt), ("wk", "bk", kt)):
                    for co in range(NCH):
                        ps = psmm.tile([128, T], F32, tag="mm")
                        for k in range(NCH):
                            nc.tensor.matmul(
                                ps[:],
                                w_sb[wname][:, k, co * 128 : (co + 1) * 128],
                                xt[:, k, :],
                                start=(k == 0),
                                stop=(k == NCH - 1),
                            )
                        if with_bias:
                            nc.scalar.activation(
                                dst[:, co, :], ps[:], Act.Identity,
                                bias=bqk_sb[bname][:, co : co + 1],
                            )
                        else:
                            nc.scalar.copy(dst[:, co, :], ps[:])

                if stage < 3:
                    continue
                # ---- per-head zero-padded Q^T (base-partition-64 matmuls
                # into shared PSUM banks crash HW; contract K=128 instead,
                # with the other head's rows zeroed on the Q side) ----
                qz = blkp.tile([128, H, T], F16, tag="qz")
                nc.gpsimd.memset(qz[:], 0.0)
                for h in range(H):
                    ch, off = divmod(h, 2)
                    off *= 64
                    nc.vector.tensor_copy(
                        qz[off : off + 64, h, :], qt[off : off + 64, ch, :]
                    )

                # ---- V projection, natural layout, interleaved with a ones
                # column per head: va[:, t, h*65:h*65+64] = V_h, [...,64] = 1 ----
                va = blkp.tile([128, BLK, H * 65], F16, tag="va")
                nc.gpsimd.memset(va[:], 1.0)
                for t in range(BLK):
                    for half in range(2):
                        psv = psmm.tile([128, 384], F32, tag="mm")
                        if with_bias:
                            nc.tensor.matmul(
                                psv[:], ones_sb[:],
                                bvo_sb["bv"][:, half * 384 : (half + 1) * 384],
                                start=True, stop=False,
                            )
                        for k in range(NCH):
                            nc.tensor.matmul(
                                psv[:],
                                xt[:, k, t * 128 : (t + 1) * 128],
                                w_sb["wv"][:, k, half * 384 : (half + 1) * 384],
                                start=(k == 0 and not with_bias),
                                stop=(k == NCH - 1),
                            )
                        dst = va[:, t, half * 390 : (half + 1) * 390]
                        dst = dst.rearrange("p (h x) -> p h x", x=65)[:, :, 0:64]
                        nc.vector.tensor_copy(
                            dst, psv[:].rearrange("p (h d) -> p h d", d=64)
                        )

                # ---- attention per column ----
                if stage < 4:
                    continue
                for t in range(BLK):
                    cg = b * BLK + t
                    # scores transposed: S_T[j, i]; exp with mask bias
                    et = colp.tile([128, H * 128], F16, tag="et")
                    for g3 in range(3):
                        pss = pssp.tile([128, 512], F32, tag="s")
                        for hh in range(4):
                            h = g3 * 4 + hh
                            ch = h // 2
                            nc.tensor.matmul(
                                pss[:, hh * 128 : (hh + 1) * 128],
                                kt[:, ch, t * 128 : (t + 1) * 128],
                                qz[:, h, t * 128 : (t + 1) * 128],
                                start=(hh == 0),
                                stop=(hh == 3),
                            )
                        nc.scalar.activation(
                            et[:, g3 * 512 : (g3 + 1) * 512], pss[:], Act.Exp,
                            bias=madd_sb[:, cg : cg + 1], scale=1.0,
                        )

                    if stage < 5:
                        continue
                    # P^T V (+ row sums in column 64 of each head block)
                    psc = []
                    for g2 in range(2):
                        pc = pscx.tile([128, 390], F32, tag="cx")
                        for hh in range(6):
                            h = g2 * 6 + hh
                            nc.tensor.matmul(
                                pc[:, hh * 65 : (hh + 1) * 65],
                                et[:, h * 128 : (h + 1) * 128],
                                va[:, t, h * 65 : (h + 1) * 65],
                                start=(hh == 0),
                                stop=(hh == 5),
                            )
                        psc.append(pc)

                    if stage < 6:
                        continue
                    # normalize: ctx / rowsum  (fp32), cast to fp16
                    recip = colp.tile([128, H], F32, tag="recip")
                    ctxn = colp.tile([128, E], F16, tag="ctxn")
                    for g2 in range(2):
                        grp = psc[g2].rearrange("p (h x) -> p h x", x=65)
                        nc.vector.reciprocal(
                            recip[:, g2 * 6 : (g2 + 1) * 6].unsqueeze(2),
                            grp[:, :, 64:65],
                        )
                        nc.vector.tensor_mul(
                            ctxn[:, g2 * 384 : (g2 + 1) * 384].rearrange(
                                "p (h d) -> p h d", d=64
                            ),
                            grp[:, :, 0:64],
                            recip[:, g2 * 6 : (g2 + 1) * 6]
                            .unsqueeze(2)
                            .broadcast_to((128, 6, 64)),
                        )

                    if stage < 7:
                        continue
                    # transpose ctx (PE) -> (e, tok) for the output projection
                    pst = pssp.tile([128, NCH, 128], F16, tag="s")
                    for ec in range(NCH):
                        nc.tensor.transpose(
                            pst[:, ec, :],
                            ctxn[:, ec * 128 : (ec + 1) * 128],
                            ident_sb[:],
                        )
                    ctxnt = colp.tile([128, NCH, 128], F16, tag="ctxnt")
                    nc.vector.tensor_copy(ctxnt[:], pst[:])

                    if stage < 8:
                        continue
                    # output projection (natural layout) and store
                    osb = colp.tile([128, E], F32, tag="osb")
                    for half in range(2):
                        po = psmm.tile([128, 384], F32, tag="mm")
                        if with_bias:
                            nc.tensor.matmul(
                                po[:], ones_sb[:],
                                bvo_sb["bo"][:, half * 384 : (half + 1) * 384],
                                start=True, stop=False,
                            )
                        for k in range(NCH):
                            nc.tensor.matmul(
                                po[:],
                                ctxnt[:, k, :],
                                w_sb["wo"][:, k, half * 384 : (half + 1) * 384],
                                start=(k == 0 and not with_bias),
                                stop=(k == NCH - 1),
                            )
                        nc.scalar.copy(osb[:, half * 384 : (half + 1) * 384], po[:])
                    nc.gpsimd.dma_start(o_d.ap()[cg], osb[:])

    nc.compile()
    return nc


_PROGRAMS = {}


def _get_program(with_bias: bool):
    if with_bias not in _PROGRAMS:
        _PROGRAMS[with_bias] = build_program(with_bias)
    return _PROGRAMS[with_bias]


def make_in_maps(x, self_attn_padding_mask, Wq, bq, Wk, bk, Wv, bv, Wo, bo,
                 with_bias):
    scaling = float(D) ** -0.5
    wq = np.ascontiguousarray((np.asarray(Wq, np.float32) * scaling).astype(np.float16))
    wk = np.ascontiguousarray(np.asarray(Wk, np.float32).astype(np.float16))
    wv = np.ascontiguousarray(np.asarray(Wv, np.float32).astype(np.float16))
    wo = np.ascontiguousarray(np.asarray(Wo, np.float32).astype(np.float16))
    mask = np.asarray(self_attn_padding_mask)[0]                   # (R, C)
    madd_full = np.where(mask, 0.0, -10000.0).astype(np.float32)   # (R, C)
    xf = np.asarray(x, np.float32)[:, :, 0, :]                     # (R, C, E)
    ident = np.eye(128, dtype=np.float16)
    in_maps = []
    for i in range(NCORES):
        cs = slice(i * CLOC, (i + 1) * CLOC)
        xs = (
            xf[:, cs]
            .transpose(1, 0, 2)                # (CLOC, R, E) tok-major
            .reshape(NTOK, NCH, 128)
            .transpose(1, 2, 0)                # (NCH, 128, NTOK) = x^T chunks
        )
        xs = np.ascontiguousarray(xs.astype(np.float16))
        m = {
            "x": xs,
            "madd": np.ascontiguousarray(madd_full[:, cs]),
            "wq": wq, "wk": wk, "wv": wv, "wo": wo,
            "ident": ident,
        }
        if with_bias:
            m["bq"] = np.ascontiguousarray(
                (np.asarray(bq, np.float32) * scaling).reshape(NCH, 128).T
            )
            m["bk"] = np.ascontiguousarray(
                np.asarray(bk, np.float32).reshape(NCH, 128).T
            )
            m["bv"] = np.asarray(bv, np.float32).astype(np.float16).reshape(1, E)
            m["bo"] = np.asarray(bo, np.float32).astype(np.float16).reshape(1, E)
        in_maps.append(m)
    return in_maps


def assemble_output(shards):
    out = np.empty((R, C, 1, E), np.float32)
    for i in range(NCORES):
        out[:, i * CLOC : (i + 1) * CLOC, 0, :] = shards[i].transpose(1, 0, 2)
    return out


def kernel(x, self_attn_padding_mask, Wq, bq, Wk, bk, Wv, bv, Wo, bo):
    global LAST_RESULTS
    with_bias = any(
        bool(np.any(np.asarray(b))) for b in (bq, bk, bv, bo)
    )
    nc = _get_program(with_bias)
    in_maps = make_in_maps(
        x, self_attn_padding_mask, Wq, bq, Wk, bk, Wv, bv, Wo, bo, with_bias
    )
    trace = os.environ.get("KERNEL_TRACE", "") not in ("", "0")
    res = run_bass_kernel_spmd(
        nc, in_maps, core_ids=list(range(NCORES)), trace=trace
    )
    LAST_RESULTS = res
    return assemble_output([res.results[i]["o"] for i in range(NCORES)])
